# revision 1
# baseline (speedup 1.0000x reference)
# CenterNet decode kernel for Trainium2 (Bass/Tile), 8-core data-parallel.
#
# Reference computation (per image):
#   heat = sigmoid(hm); heat *= (3x3 maxpool(heat) == heat)    # pool NMS
#   conf = max_c heat; cls = argmax_c heat
#   boxes from wh/reg + meshgrid; dets = [x1,y1,x2,y2,conf,cls] * (conf > 0.3)
#
# Device algorithm works in logit space (sigmoid is strictly monotone, so
# pooling / suppression / channel-argmax commute with it; one sigmoid at the
# end on the per-pixel winner):
#   phase 1 (channels on partitions, strip of rows in free dim):
#     pooled = separable 3x3 max (2 horizontal + 2 vertical shifted maxes)
#     d = pooled - x              (>= 0; == 0 iff local max)
#     w = x - 1e12*d              (exact x at local maxima, huge-negative else)
#   phase 2 (per 128-pixel chunk, PE-transpose to [pixel, C]):
#     conf = reduce_max_c(w)      (exact winner logit)
#     eq   = (w == conf); idx = reduce_min_c(iota - 1e6*eq)   (first-index rule)
#   assembly (pixels on partitions):
#     conf_s = sigmoid(conf); mask = conf_s > 0.3
#     dets columns from wh/reg/meshgrid, all multiplied by mask.
import os
import sys
import threading

for _p in ("/opt/trn_rl_repo", "/root/.axon_site/_ro/trn_rl_repo"):
    if os.path.isdir(_p) and _p not in sys.path:
        sys.path.insert(0, _p)

import numpy as np

from concourse import bacc, bass, masks, mybir, tile

F32 = mybir.dt.float32
I32 = mybir.dt.int32
AX = mybir.AxisListType
OP = mybir.AluOpType
ACTF = mybir.ActivationFunctionType

NEG = -1e30     # pad value (acts as -inf for maxes)
BIGM = 1e12     # suppression multiplier
BIGI = 1e6      # argmax index offset (c - BIGI exact in f32 for c < 2^19-ish)

_FLT_MAX = float(np.finfo(np.float32).max)
_CUSTOM = {}


def _custom_ops():
    """Register (once per process) the two fused DVE micro-ops:
    CN_WSEL:  w = x if x == pooled else -FLT_MAX       (suppression)
    CN_IDXC:  cand = (channel pos) if w == conf else s0  (argmax candidates,
              channel pos = Idx - SubIdx*s1 within [P, S, N] pages)"""
    if _CUSTOM:
        return _CUSTOM
    import re
    from concourse.dve_spec import (Spec, Src0, Src1, MaxNeg, select, eq,
                                    Idx, SubIdx, C0, C1)
    from concourse import dve_ops as D
    from concourse.dve_ops import DveOp, OPS

    def reg(name, spec, subdim):
        for op in OPS:
            if op.name == name:
                return op
        op = DveOp(name, spec, subdim=subdim, uops_sha={})
        OPS.append(op)
        D.CUSTOM_DVE_SPECS[name] = spec
        D._SUB_OPCODE_FOR_NAME[name] = D._CUSTOM_DVE_ROW_BASE + len(OPS) - 1
        for ver in ("v3", "v4"):
            try:
                op.compile(ver)
            except ValueError as e:
                m = re.search(r"%s: ([0-9a-f]+)" % ver, str(e))
                if m:
                    op.uops_sha[ver] = m.group(1)
                    op.compile(ver)
        return op

    _CUSTOM["wsel"] = reg(
        "CN_WSEL",
        Spec(body=select(eq(Src0, Src1), Src0, MaxNeg),
             reference=lambda in0, in1, c0=0, c1=0, c2=0: np.where(
                 in0 == in1.reshape(in0.shape), in0,
                 -_FLT_MAX).astype(in0.dtype)),
        subdim=False)
    _CUSTOM["idxc"] = reg(
        "CN_IDXC",
        Spec(body=select(eq(Src0, Src1), Idx - SubIdx * C1, C0),
             reference=lambda in0, in1, c0=0, c1=0, c2=0: np.where(
                 in0 == in1.reshape(in0.shape),
                 (np.arange(in0.shape[-2] * in0.shape[-1], dtype=np.float32)
                  .reshape(in0.shape[-2], in0.shape[-1])
                  - np.arange(in0.shape[-2], dtype=np.float32)[:, None]
                  * np.float32(np.asarray(c1).flat[0]))[None],
                 np.asarray(c0, dtype=np.float32)).astype(np.float32)),
        subdim=True)
    return _CUSTOM


def build_nc(Bc=4, C=80, H=256, W=256, S=16, n_devices=8, reps=1):
    """Build the per-core program: inputs hm [Bc,C,H,W], wh/reg [Bc,2,H,W],
    xyv [2,H,W] (meshgrid/W consts), output dets [Bc, H*W, 6]."""
    assert H % S == 0 and (S * W) % 128 == 0 and (H * W) % 128 == 0
    n_strips = H // S
    cps = (S * W) // 128          # 128-pixel chunks per strip
    G = (H * W) // 128            # pixels per partition in assembly layout
    TPG = min(4, cps)             # chunks per PSUM tile
    assert TPG * C * 4 <= 2048 and cps % TPG == 0
    PPS = (S * W) // G            # assembly partitions covered by one strip
    assert PPS >= 1

    cust = _custom_ops()
    nc = bacc.Bacc("TRN2", target_bir_lowering=False, debug=False,
                   num_devices=n_devices)
    hm = nc.dram_tensor("hm", [Bc, C, H, W], F32, kind="ExternalInput")
    wh = nc.dram_tensor("wh", [Bc, 2, H * W], F32, kind="ExternalInput")
    reg = nc.dram_tensor("reg", [Bc, 2, H * W], F32, kind="ExternalInput")
    xyv = nc.dram_tensor("xyv", [2, H * W], F32, kind="ExternalInput")
    dets = nc.dram_tensor("dets", [Bc, H * W, 6], F32, kind="ExternalOutput")

    Wp = W + 2
    with tile.TileContext(nc) as tc:
        with (
            tc.tile_pool(name="singles", bufs=1) as singles,
            tc.tile_pool(name="xp", bufs=2) as xp_pool,
            tc.tile_pool(name="pool_tmp", bufs=3) as tmp_pool,
            tc.tile_pool(name="wv", bufs=2) as w_pool,
            tc.tile_pool(name="ph2", bufs=6) as ph2_pool,
            tc.tile_pool(name="strip_res", bufs=4) as sres_pool,
            tc.tile_pool(name="imgbuf", bufs=2) as img_pool,
            tc.tile_pool(name="asm", bufs=1) as asm_pool,
            tc.tile_pool(name="psum_t", bufs=6, space="PSUM") as psum_pool,
            tc.tile_pool(name="psum_b", bufs=2, space="PSUM") as psum_b_pool,
        ):
            ident = singles.tile([128, 128], F32)
            masks.make_identity(nc, ident[:])
            ident_c = singles.tile([C, C], F32)
            masks.make_identity(nc, ident_c[:])

            # meshgrid constants, already divided by W/H: [128, G] each
            xvn = singles.tile([128, G], F32)
            yvn = singles.tile([128, G], F32)
            nc.sync.dma_start(xvn[:], xyv[0].rearrange("(p g) -> p g", p=128))
            nc.sync.dma_start(yvn[:], xyv[1].rearrange("(p g) -> p g", p=128))

            for _rep in range(reps):
              for b in range(Bc):
                conf_g = img_pool.tile([128, G], F32, tag="conf_g")
                idx_g = img_pool.tile([128, G], F32, tag="idx_g")

                for s in range(n_strips):
                    r0 = s * S
                    # --- load strip with 1-row halo, padded W -------------
                    xp = xp_pool.tile([C, (S + 2) * Wp], F32, tag="xp")
                    xp3 = xp[:].rearrange("c (r w) -> c r w", w=Wp)
                    # pad columns 0 and W+1 of every row
                    nc.gpsimd.memset(
                        xp3[:, :, 0:Wp:(Wp - 1)], NEG)
                    lo = max(r0 - 1, 0)
                    hi = min(r0 + S + 1, H)
                    dst_r0 = 1 - (r0 - lo)
                    nc.sync.dma_start(
                        xp3[:, dst_r0:dst_r0 + (hi - lo), 1:W + 1],
                        hm[b, :, lo:hi, :])
                    if r0 == 0:
                        nc.gpsimd.memset(xp3[:, 0, 1:W + 1], NEG)
                    if r0 + S == H:
                        nc.gpsimd.memset(xp3[:, S + 1, 1:W + 1], NEG)

                    # --- separable 3x3 max pool ---------------------------
                    m1 = tmp_pool.tile([C, (S + 2) * (W + 1)], F32, tag="pt")
                    m13 = m1[:].rearrange("c (r w) -> c r w", w=W + 1)
                    nc.vector.tensor_tensor(
                        m13[:, :, :], xp3[:, :, 0:W + 1], xp3[:, :, 1:W + 2],
                        op=OP.max)
                    hx = tmp_pool.tile([C, (S + 2) * W], F32, tag="pt")
                    hx3 = hx[:].rearrange("c (r w) -> c r w", w=W)
                    nc.vector.tensor_tensor(
                        hx3[:, :, :], m13[:, :, 0:W], m13[:, :, 1:W + 1],
                        op=OP.max)
                    mv = tmp_pool.tile([C, (S + 1) * W], F32, tag="pt")
                    mv3 = mv[:].rearrange("c (r w) -> c r w", w=W)
                    nc.vector.tensor_tensor(
                        mv3[:, :, :], hx3[:, 0:S + 1, :], hx3[:, 1:S + 2, :],
                        op=OP.max)
                    vm = tmp_pool.tile([C, S * W], F32, tag="pt")
                    vm3 = vm[:].rearrange("c (r w) -> c r w", w=W)
                    nc.vector.tensor_tensor(
                        vm3[:, :, :], mv3[:, 0:S, :], mv3[:, 1:S + 1, :],
                        op=OP.max)

                    # --- suppression: w = x if x == pooled else -FLT_MAX --
                    xr = xp3[:, 1:S + 1, 1:W + 1]
                    wv = w_pool.tile([C, S * W], F32, tag="wv")
                    wv3 = wv[:].rearrange("c (r w) -> c r w", w=W)
                    nc.vector._custom_dve(cust["wsel"], out=wv3[:, :, :],
                                          in0=xr, in1=vm3[:, :, :])

                    # --- phase 2: transpose chunks, reduce over channels --
                    conf_t = sres_pool.tile([128, cps], F32, tag="conf_t")
                    idxm_t = sres_pool.tile([128, cps], F32, tag="idxm_t")
                    for g0 in range(0, cps, TPG):
                        wt = psum_pool.tile([128, TPG * C], F32, tag="wt")
                        wt3 = wt[:].rearrange("p (t c) -> p t c", c=C)
                        for t in range(TPG):
                            k = g0 + t
                            nc.tensor.transpose(
                                wt3[:, t, :].rearrange("p c -> p c"),
                                wv[:, k * 128:(k + 1) * 128],
                                ident_c[:])
                        nc.vector.tensor_reduce(
                            conf_t[:, g0:g0 + TPG], wt3[:, :, :],
                            axis=AX.X, op=OP.max)
                        cb = conf_t[:, g0:g0 + TPG].unsqueeze(-1) \
                            .broadcast_to((128, TPG, C))
                        im = ph2_pool.tile([128, TPG * C], F32, tag="im")
                        im3 = im[:].rearrange("p (t c) -> p t c", c=C)
                        nc.vector._custom_dve(cust["idxc"], out=im3[:, :, :],
                                              in0=wt3[:, :, :], in1=cb,
                                              s0=1e4, s1=float(C))
                        nc.vector.tensor_reduce(
                            idxm_t[:, g0:g0 + TPG], im3[:, :, :],
                            axis=AX.X, op=OP.min)

                    # --- transpose back to pixel-linear rows --------------
                    ct_ps = psum_b_pool.tile([cps, 128], F32, tag="tb")
                    nc.tensor.transpose(ct_ps[:], conf_t[:], ident[:])
                    conf_lin = sres_pool.tile([cps, 128], F32, tag="conf_lin")
                    nc.scalar.copy(conf_lin[:], ct_ps[:])
                    it_ps = psum_b_pool.tile([cps, 128], F32, tag="tb")
                    nc.tensor.transpose(it_ps[:], idxm_t[:], ident[:])
                    idx_lin = sres_pool.tile([cps, 128], F32, tag="idx_lin")
                    nc.scalar.copy(idx_lin[:], it_ps[:])

                    # --- scatter strip rows into per-image [128, G] -------
                    p0 = (s * S * W) // G
                    nc.sync.dma_start(
                        conf_g[p0:p0 + PPS, :] if PPS > 1 else
                        conf_g[p0:p0 + 1, :],
                        conf_lin[:].rearrange("a b -> a b"))
                    nc.sync.dma_start(
                        idx_g[p0:p0 + PPS, :] if PPS > 1 else
                        idx_g[p0:p0 + 1, :],
                        idx_lin[:].rearrange("a b -> a b"))

                # --- assembly for image b (pixels on partitions) ----------
                wh0 = asm_pool.tile([128, G], F32, tag="wh0")
                wh1 = asm_pool.tile([128, G], F32, tag="wh1")
                rg0 = asm_pool.tile([128, G], F32, tag="rg0")
                rg1 = asm_pool.tile([128, G], F32, tag="rg1")
                nc.sync.dma_start(wh0[:], wh[b, 0].rearrange("(p g) -> p g", p=128))
                nc.sync.dma_start(wh1[:], wh[b, 1].rearrange("(p g) -> p g", p=128))
                nc.sync.dma_start(rg0[:], reg[b, 0].rearrange("(p g) -> p g", p=128))
                nc.sync.dma_start(rg1[:], reg[b, 1].rearrange("(p g) -> p g", p=128))

                confs = asm_pool.tile([128, G], F32, tag="confs")
                nc.scalar.activation(confs[:], conf_g[:], ACTF.Sigmoid)
                mask = asm_pool.tile([128, G], F32, tag="mask")
                nc.vector.tensor_scalar(mask[:], confs[:], 0.3, None,
                                        op0=OP.is_gt)

                out_img = asm_pool.tile([128, G * 6], F32, tag="out_img")
                o3 = out_img[:].rearrange("p (g k) -> p g k", k=6)

                # masked center coords and half-extents
                tcx = asm_pool.tile([128, G], F32, tag="tcx")
                nc.vector.scalar_tensor_tensor(tcx[:], rg0[:], 1.0 / W, xvn[:],
                                               op0=OP.mult, op1=OP.add)
                tcy = asm_pool.tile([128, G], F32, tag="tcy")
                nc.vector.scalar_tensor_tensor(tcy[:], rg1[:], 1.0 / H, yvn[:],
                                               op0=OP.mult, op1=OP.add)
                nc.vector.tensor_tensor(tcx[:], tcx[:], mask[:], op=OP.mult)
                nc.vector.tensor_tensor(tcy[:], tcy[:], mask[:], op=OP.mult)
                hwx = asm_pool.tile([128, G], F32, tag="hwx")
                nc.vector.scalar_tensor_tensor(hwx[:], wh0[:], 0.5 / W, mask[:],
                                               op0=OP.mult, op1=OP.mult)
                hwy = asm_pool.tile([128, G], F32, tag="hwy")
                nc.vector.scalar_tensor_tensor(hwy[:], wh1[:], 0.5 / H, mask[:],
                                               op0=OP.mult, op1=OP.mult)

                nc.vector.tensor_tensor(o3[:, :, 0], tcx[:], hwx[:], op=OP.subtract)
                nc.vector.tensor_tensor(o3[:, :, 1], tcy[:], hwy[:], op=OP.subtract)
                nc.vector.tensor_tensor(o3[:, :, 2], tcx[:], hwx[:], op=OP.add)
                nc.vector.tensor_tensor(o3[:, :, 3], tcy[:], hwy[:], op=OP.add)
                nc.vector.tensor_tensor(o3[:, :, 4], confs[:], mask[:], op=OP.mult)
                nc.vector.tensor_tensor(o3[:, :, 5], idx_g[:], mask[:], op=OP.mult)

                nc.sync.dma_start(
                    dets[b].rearrange("(p g) k -> p (g k)", p=128), out_img[:])

    nc.compile()
    return nc


_CACHE = {}
_CACHE_LOCK = threading.Lock()


def _get_nc(key, **kw):
    with _CACHE_LOCK:
        if key not in _CACHE:
            _CACHE[key] = build_nc(**kw)
        return _CACHE[key]


def _xyv(H, W):
    yv, xv = np.meshgrid(np.arange(H, dtype=np.float32),
                         np.arange(W, dtype=np.float32), indexing="ij")
    return np.stack([xv / W, yv / H]).reshape(2, H * W).astype(np.float32)


def kernel(hm: np.ndarray, wh: np.ndarray, reg: np.ndarray) -> np.ndarray:
    from concourse.bass_utils import run_bass_kernel_spmd

    B, C, H, W = hm.shape
    n_cores = 8
    assert B % n_cores == 0
    Bc = B // n_cores
    nc = _get_nc(("full", Bc, C, H, W), Bc=Bc, C=C, H=H, W=W, S=16)
    xyv = _xyv(H, W)
    in_maps = []
    for i in range(n_cores):
        sl = slice(i * Bc, (i + 1) * Bc)
        in_maps.append({
            "hm": np.ascontiguousarray(hm[sl]),
            "wh": np.ascontiguousarray(wh[sl]).reshape(Bc, 2, H * W),
            "reg": np.ascontiguousarray(reg[sl]).reshape(Bc, 2, H * W),
            "xyv": xyv,
        })
    res = run_bass_kernel_spmd(nc, in_maps, core_ids=list(range(n_cores)))
    return np.concatenate([res.results[i]["dets"] for i in range(n_cores)],
                          axis=0)



# revision 20
# speedup vs baseline: 39.0809x; 39.0809x over previous
# CenterNet decode kernel for Trainium2 (Bass/Tile), 8-core data-parallel.
#
# Reference computation (per image):
#   heat = sigmoid(hm); heat *= (3x3 maxpool(heat) == heat)    # pool NMS
#   conf = max_c heat; cls = argmax_c heat
#   boxes from wh/reg + meshgrid; dets = [x1,y1,x2,y2,conf,cls] * (conf > 0.3)
#
# Device algorithm works in logit space (sigmoid strictly monotone, so
# pooling / suppression / channel-argmax commute with it; one sigmoid at the
# end on the per-pixel winner):
#
# phase 1 — plane-strip tiles. The (b, s, c) plane-strip jobs (b image, s
#   row-strip of S rows, c channel) are packed 128 per SBUF tile so every
#   partition is busy (Bc*NS*C = 5120 jobs -> 40 exactly-full tiles).
#   Per tile: load rows with 1-row halo into a column-padded f32 buffer,
#   separable 3x3 max (vertical twice, then horizontal twice — shrinking
#   rows first), then one fused custom DVE op:
#     wq = x + OFF  if x == pooled   else 0        (exact f32 equality)
#   OFF=32 makes every surviving logit positive (|logit| < 16 whp), so the
#   suppressed 0 can never win the channel max unless all channels are
#   suppressed — and then sigmoid(0 - OFF) ~ 0 < 0.3 masks the pixel.
#   The 4 max passes are split column-wise between DVE and GPSIMD (Pool)
#   so both engines stream concurrently; the custom select is DVE-only.
#
# phase 2 — per (b, s) group (80 channel-planes, contiguous partitions
#   possibly spanning two tiles): PE-transpose 128-pixel chunks into
#   bank-aligned PSUM slots (12 chunks per 3-bank PSUM tile, slot stride
#   128 f32 so no chunk crosses a bank). reduce_max over channels -> conf;
#   argmax by exact equality (custom idx-candidate op) + reduce_min.
#   Results accumulate into per-image [128, G] slabs.
#
# per image — batched transpose-back of the slabs to pixel-block-major
#   layout, then assembly: conf_s = sigmoid(conf - OFF) on ACT (bias),
#   mask = conf_s > 0.3, box columns from wh/reg/meshgrid, all masked.
import os
import sys
import threading

for _p in ("/opt/trn_rl_repo", "/root/.axon_site/_ro/trn_rl_repo"):
    if os.path.isdir(_p) and _p not in sys.path:
        sys.path.insert(0, _p)

import numpy as np

from concourse import bacc, bass, masks, mybir, tile

F32 = mybir.dt.float32
AX = mybir.AxisListType
OP = mybir.AluOpType
ACTF = mybir.ActivationFunctionType

NEG = -1e30     # pad value (acts as -inf for maxes)
OFF = 32.0      # logit shift: |logit| < 16 whp, so logit+OFF in (16, 48)
BIGI = 1e4      # idxc filler for non-matching lanes (> any channel index)

_FLT_MAX = float(np.finfo(np.float32).max)
_CUSTOM = {}


def _custom_ops():
    """Register (once per process) the two fused DVE micro-ops:
    CN_WSEL3: wq = (x + c0) if x == pooled else 0     (suppress + shift)
    CN_IDXC:  cand = (channel pos) if w == conf else s0  (argmax candidates,
              channel pos = Idx - SubIdx*s1 within [P, S, N] pages)"""
    if _CUSTOM:
        return _CUSTOM
    import re
    from concourse.dve_spec import (Spec, Src0, Src1, Zero, select, eq,
                                    Idx, SubIdx, C0, C1, scan, AluOp)
    from concourse import dve_ops as D
    from concourse.dve_ops import DveOp, OPS

    def reg(name, spec, subdim):
        for op in OPS:
            if op.name == name:
                return op
        op = DveOp(name, spec, subdim=subdim, uops_sha={})
        OPS.append(op)
        D.CUSTOM_DVE_SPECS[name] = spec
        D._SUB_OPCODE_FOR_NAME[name] = D._CUSTOM_DVE_ROW_BASE + len(OPS) - 1
        for ver in ("v3", "v4"):
            try:
                op.compile(ver)
            except ValueError as e:
                m = re.search(r"%s: ([0-9a-f]+)" % ver, str(e))
                if m:
                    op.uops_sha[ver] = m.group(1)
                    op.compile(ver)
        return op

    _CUSTOM["wsel3"] = reg(
        "CN_WSEL3",
        Spec(body=select(eq(Src0, Src1), Src0 + C0, Zero),
             reference=lambda in0, in1, c0=0, c1=0, c2=0: np.where(
                 in0 == in1.reshape(in0.shape),
                 in0 + np.float32(np.asarray(c0).flat[0]),
                 np.float32(0.0)).astype(in0.dtype)),
        subdim=False)
    _CUSTOM["idxc"] = reg(
        "CN_IDXC",
        Spec(body=select(eq(Src0, Src1), Idx - SubIdx * C1, C0),
             reference=lambda in0, in1, c0=0, c1=0, c2=0: np.where(
                 in0 == in1.reshape(in0.shape),
                 (np.arange(in0.shape[-2] * in0.shape[-1], dtype=np.float32)
                  .reshape(in0.shape[-2], in0.shape[-1])
                  - np.arange(in0.shape[-2], dtype=np.float32)[:, None]
                  * np.float32(np.asarray(c1).flat[0]))[None],
                 np.asarray(c0, dtype=np.float32)).astype(np.float32)),
        subdim=True)
    return _CUSTOM


def build_nc(Bc=4, C=80, H=256, W=256, S=16, n_devices=8, reps=1,
             pool_frac=0.0, TCH=12, stage=3):
    # pool_frac > 0 is unusable on real HW: the Pool/GPSIMD engine fails the
    # compiler's ISA check for TensorTensor max (only arithmetic ops pass).
    # stage: debug bisect — 1 = phase1 only, 2 = +phase2, 3 = full.
    """Per-core program. Inputs: hm [Bc,C,H,W], wh/reg [Bc,2,H*W],
    xyv [2,H*W] (meshgrid/W consts); output dets [Bc, H*W, 6]."""
    NS = H // S                   # strips per image
    NJ = Bc * NS * C              # plane-strip jobs
    assert NJ % 128 == 0
    NT = NJ // 128                # full 128-partition tiles
    NG = Bc * NS                  # (b, s) groups of C consecutive jobs
    CPS = (S * W) // 128          # 128-pixel chunks per group
    G = (H * W) // 128            # assembly free size per image
    TPI = NT // Bc                # tiles per image (plane count Bc*NS*C/Bc
    assert (NS * C) % 128 == 0    #  divisible -> images end on tile bounds)
    Wp = W + 2
    R = S + 2                     # rows incl. halo

    cust = _custom_ops()
    nc = bacc.Bacc("TRN2", target_bir_lowering=False, debug=False,
                   num_devices=n_devices)
    hm = nc.dram_tensor("hm", [Bc, C, H, W], F32, kind="ExternalInput")
    wh = nc.dram_tensor("wh", [Bc, 2, H * W], F32, kind="ExternalInput")
    reg = nc.dram_tensor("reg", [Bc, 2, H * W], F32, kind="ExternalInput")
    xyv = nc.dram_tensor("xyv", [2, H * W], F32, kind="ExternalInput")
    dets = nc.dram_tensor("dets", [Bc, H * W, 6], F32, kind="ExternalOutput")

    # column split points for the DVE/Pool work division of the max passes
    def split(n):
        d = n - int(round(n * pool_frac))
        return max(0, min(n, d))

    with tile.TileContext(nc) as tc:
        with (
            tc.tile_pool(name="singles", bufs=1) as singles,
            tc.tile_pool(name="xp", bufs=2) as xp_pool,
            tc.tile_pool(name="tmp", bufs=1) as tmp_pool,
            tc.tile_pool(name="vm", bufs=1) as vm_pool,
            tc.tile_pool(name="wq", bufs=2) as wq_pool,
            tc.tile_pool(name="stage", bufs=1) as stage_pool,
            tc.tile_pool(name="im", bufs=2) as im_pool,
            tc.tile_pool(name="slab", bufs=2) as slab_pool,
            tc.tile_pool(name="tr", bufs=2) as tr_pool,
            tc.tile_pool(name="asm", bufs=1) as asm_pool,
            tc.tile_pool(name="outb", bufs=1) as out_pool,
            tc.tile_pool(name="ps2", bufs=2, space="PSUM") as ps2_pool,
            tc.tile_pool(name="psb", bufs=2, space="PSUM") as psb_pool,
        ):
            ident = singles.tile([128, 128], F32)
            masks.make_identity(nc, ident[:])
            negoff = singles.tile([128, 1], F32)
            nc.gpsimd.memset(negoff[:], -OFF)

            # meshgrid constants, already divided by W/H: [128, G] each
            xvn = singles.tile([128, G], F32)
            yvn = singles.tile([128, G], F32)
            nc.sync.dma_start(xvn[:], xyv[0].rearrange("(p g) -> p g", p=128))
            nc.sync.dma_start(yvn[:], xyv[1].rearrange("(p g) -> p g", p=128))

            for _rep in range(reps):
              wq_tiles = {}          # tile idx -> wq tile (ring of 3)
              slabs = {}             # b -> (conf_slab, idx_slab)

              for t in range(NT):
                j0 = t * 128                       # first job in tile
                # ---- groups intersecting this tile --------------------
                g0 = j0 // C
                g1 = (j0 + 127) // C
                subs = []                          # (pa, pb, b, s, ca)
                for g in range(g0, g1 + 1):
                    pa = max(0, g * C - j0)
                    pb = min(128, (g + 1) * C - j0)
                    b, s = divmod(g, NS)
                    subs.append((pa, pb, b, s, (j0 + pa) - g * C))

                if t % TPI == 0:
                    b_now = t // TPI
                    cs = slab_pool.tile([128, G], F32, tag="conf_slab")
                    isl = slab_pool.tile([128, G], F32, tag="idx_slab")
                    slabs[b_now] = (cs, isl)

                # ---- load strip rows (halo'd) into padded f32 tile ----
                xp = xp_pool.tile([128, R * Wp], F32, tag="xp")
                xp3 = xp[:].rearrange("p (r w) -> p r w", w=Wp)
                nc.gpsimd.memset(xp3[:, :, 0:Wp:(Wp - 1)], NEG)
                for (pa, pb, b, s, ca) in subs:
                    r0 = s * S
                    lo = max(r0 - 1, 0)
                    hi = min(r0 + S + 1, H)
                    dr0 = 1 - (r0 - lo)
                    nc.sync.dma_start(
                        xp3[pa:pb, dr0:dr0 + (hi - lo), 1:W + 1],
                        hm[b, ca:ca + (pb - pa), lo:hi, :])
                    # duplicate boundary rows into the halo slots: for 3x3
                    # max, max(x0,x0,x1) == max(x0,x1), so this equals the
                    # reference's -inf SAME padding.
                    if r0 == 0:
                        nc.sync.dma_start(
                            xp3[pa:pb, 0, 1:W + 1],
                            hm[b, ca:ca + (pb - pa), 0, :])
                    if r0 + S == H:
                        nc.sync.dma_start(
                            xp3[pa:pb, R - 1, 1:W + 1],
                            hm[b, ca:ca + (pb - pa), H - 1, :])

                # ---- separable 3x3 max: V, V, H, H (f32, split DVE/Pool)
                v1 = tmp_pool.tile([128, (R - 1) * Wp], F32, tag="v1")
                v13 = v1[:].rearrange("p (r w) -> p r w", w=Wp)
                cd = split(Wp)
                if cd > 0:
                    nc.vector.tensor_tensor(
                        v13[:, :, 0:cd], xp3[:, 0:R - 1, 0:cd],
                        xp3[:, 1:R, 0:cd], op=OP.max)
                if cd < Wp:
                    nc.gpsimd.tensor_tensor(
                        v13[:, :, cd:Wp], xp3[:, 0:R - 1, cd:Wp],
                        xp3[:, 1:R, cd:Wp], op=OP.max)

                v2 = tmp_pool.tile([128, S * Wp], F32, tag="v2")
                v23 = v2[:].rearrange("p (r w) -> p r w", w=Wp)
                if cd > 0:
                    nc.vector.tensor_tensor(
                        v23[:, :, 0:cd], v13[:, 0:S, 0:cd],
                        v13[:, 1:S + 1, 0:cd], op=OP.max)
                if cd < Wp:
                    nc.gpsimd.tensor_tensor(
                        v23[:, :, cd:Wp], v13[:, 0:S, cd:Wp],
                        v13[:, 1:S + 1, cd:Wp], op=OP.max)

                h1 = tmp_pool.tile([128, S * (W + 1)], F32, tag="v1")
                h13 = h1[:].rearrange("p (r w) -> p r w", w=W + 1)
                hd = split(W + 1)
                if hd > 0:
                    nc.vector.tensor_tensor(
                        h13[:, :, 0:hd], v23[:, :, 0:hd],
                        v23[:, :, 1:hd + 1], op=OP.max)
                if hd < W + 1:
                    nc.gpsimd.tensor_tensor(
                        h13[:, :, hd:W + 1], v23[:, :, hd:W + 1],
                        v23[:, :, hd + 1:W + 2], op=OP.max)

                vm = vm_pool.tile([128, S * W], F32, tag="vm")
                vm3 = vm[:].rearrange("p (r w) -> p r w", w=W)
                vd = split(W)
                if vd > 0:
                    nc.vector.tensor_tensor(
                        vm3[:, :, 0:vd], h13[:, :, 0:vd],
                        h13[:, :, 1:vd + 1], op=OP.max)
                if vd < W:
                    nc.gpsimd.tensor_tensor(
                        vm3[:, :, vd:W], h13[:, :, vd:W],
                        h13[:, :, vd + 1:W + 1], op=OP.max)

                # ---- fused suppress + shift: wq = x+OFF if max else 0 -
                wq = wq_pool.tile([128, S * W], F32, tag="wq")
                wq3 = wq[:].rearrange("p (r w) -> p r w", w=W)
                nc.vector._custom_dve(
                    cust["wsel3"], out=wq3[:, :, :],
                    in0=xp3[:, 1:S + 1, 1:W + 1], in1=vm3[:, :, :], s0=OFF)
                wq_tiles[t] = wq

                # ---- phase 2 for every group that completes at tile t -
                for g in (range(g0, g1 + 1) if stage >= 1.25 else ()):
                    if ((g + 1) * C - 1) // 128 != t:
                        continue
                    b, s = divmod(g, NS)
                    cs, isl = slabs[b]
                    # partition sub-ranges of this group's planes
                    parts = []
                    for tt in range((g * C) // 128, t + 1):
                        pa = max(0, g * C - tt * 128)
                        pb = min(128, (g + 1) * C - tt * 128)
                        parts.append((tt, pa, pb, (tt * 128 + pa) - g * C))
                    if len(parts) > 1:
                        # Split groups: stage the two partition ranges into a
                        # contiguous base-0 tile (SBUF-SBUF DMA). Mixed PE
                        # tile positions writing one PSUM slot hang real HW,
                        # so every transpose must be single-part from base 0.
                        st = stage_pool.tile([C, S * W], F32, tag="st")
                        off = 0
                        for (tt, pa, pb, cc) in parts:
                            nc.sync.dma_start(
                                st[off:off + (pb - pa), :],
                                wq_tiles[tt][pa:pb, :])
                            off += pb - pa
                        parts = [(-1, 0, C, 0)]
                        src_map = {-1: st}
                    else:
                        src_map = {parts[0][0]: wq_tiles[parts[0][0]]}
                    for k0 in range(0, CPS, TCH):
                        kn = min(TCH, CPS - k0)
                        wt = ps2_pool.tile([128, TCH * 128], F32, tag="wt")
                        for ki in range(kn):
                            k = k0 + ki
                            for (tt, pa, pb, cc) in parts:
                                # PE quadrant rule: operand base must be a
                                # valid tile position for its span (<=32 ->
                                # 0/32/64/96, <=64 -> 0/64, else 0). Round
                                # the base down; the junk columns land in
                                # the 48-col margin before the channel area.
                                for qa in (64, 32, 0):
                                    w = pb - qa
                                    if (qa <= pa and pa - qa <= 48 and
                                        (qa == 0 or (qa == 64 and w <= 64)
                                         or (qa == 32 and w <= 32))):
                                        break
                                o0 = ki * 128 + 48 + cc - (pa - qa)
                                nc.tensor.transpose(
                                    wt[:, o0:o0 + (pb - qa)],
                                    src_map[tt][qa:pb, k * 128:(k + 1) * 128],
                                    ident[qa:pb, qa:qa + (pb - qa)])
                        if stage < 1.5:
                            continue
                        wt3 = wt[:].rearrange(
                            "p (t x) -> p t x", x=128)[:, 0:kn, 48:48 + C]
                        cols = slice(s * CPS + k0, s * CPS + k0 + kn)
                        nc.vector.tensor_reduce(
                            cs[:, cols], wt3, axis=AX.X, op=OP.max)
                        if stage < 1.75:
                            continue
                        cb = cs[:, cols].unsqueeze(-1) \
                            .broadcast_to((128, kn, C))
                        im = im_pool.tile([128, TCH * C], F32, tag="im")
                        im3 = im[:].rearrange("p (t c) -> p t c", c=C)[:, 0:kn]
                        nc.vector._custom_dve(cust["idxc"], out=im3,
                                              in0=wt3, in1=cb,
                                              s0=BIGI, s1=float(C))
                        if stage < 2:
                            continue
                        nc.vector.tensor_reduce(
                            isl[:, cols], im3, axis=AX.X, op=OP.min)

                # ---- assembly when an image's groups all finished -----
                if (t + 1) % TPI != 0:
                    continue
                b = t // TPI
                cs, isl = slabs.pop(b)
                if stage < 2.5:
                    zout = out_pool.tile([128, G * 6], F32, tag="out_img")
                    nc.gpsimd.memset(zout[:], 0.0)
                    nc.sync.dma_start(
                        dets[b].rearrange("(p g) k -> p (g k)", p=128),
                        zout[:])
                    continue

                # transpose slabs back to pixel-block-major [128, G]
                cg = asm_pool.tile([128, G], F32, tag="cg")
                ig = asm_pool.tile([128, G], F32, tag="ig")
                for src, dst in ((cs, cg), (isl, ig)):
                    tb = psb_pool.tile([128, G], F32, tag="tb")
                    for blk in range(G // 128):
                        nc.tensor.transpose(
                            tb[:, blk * 128:(blk + 1) * 128],
                            src[:, blk * 128:(blk + 1) * 128], ident[:])
                    tr = tr_pool.tile([128, G], F32, tag="tr")
                    nc.scalar.copy(tr[:], tb[:])
                    QB = (128 * 128) // G   # dst partitions per 128-chunk blk
                    for blk in range(G // 128):
                        nc.sync.dma_start(
                            dst[QB * blk:QB * (blk + 1), :],
                            tr[:, blk * 128:(blk + 1) * 128])

                wh0 = asm_pool.tile([128, G], F32, tag="wh0")
                wh1 = asm_pool.tile([128, G], F32, tag="wh1")
                rg0 = asm_pool.tile([128, G], F32, tag="rg0")
                rg1 = asm_pool.tile([128, G], F32, tag="rg1")
                nc.sync.dma_start(wh0[:], wh[b, 0].rearrange("(p g) -> p g", p=128))
                nc.sync.dma_start(wh1[:], wh[b, 1].rearrange("(p g) -> p g", p=128))
                nc.sync.dma_start(rg0[:], reg[b, 0].rearrange("(p g) -> p g", p=128))
                nc.sync.dma_start(rg1[:], reg[b, 1].rearrange("(p g) -> p g", p=128))

                confs = asm_pool.tile([128, G], F32, tag="confs")
                nc.scalar.activation(confs[:], cg[:], ACTF.Sigmoid,
                                     bias=negoff[:])
                mask = asm_pool.tile([128, G], F32, tag="mask")
                nc.vector.tensor_scalar(mask[:], confs[:], 0.3, None,
                                        op0=OP.is_gt)

                out_img = out_pool.tile([128, G * 6], F32, tag="out_img")
                o3 = out_img[:].rearrange("p (g k) -> p g k", k=6)

                # masked center coords and masked half-extents
                tcx = asm_pool.tile([128, G], F32, tag="tcx")
                nc.vector.scalar_tensor_tensor(tcx[:], rg0[:], 1.0 / W, xvn[:],
                                               op0=OP.mult, op1=OP.add)
                tcxm = asm_pool.tile([128, G], F32, tag="tcxm")
                nc.vector.tensor_tensor(tcxm[:], tcx[:], mask[:], op=OP.mult)
                tcy = asm_pool.tile([128, G], F32, tag="tcy")
                nc.vector.scalar_tensor_tensor(tcy[:], rg1[:], 1.0 / H, yvn[:],
                                               op0=OP.mult, op1=OP.add)
                tcym = asm_pool.tile([128, G], F32, tag="tcym")
                nc.vector.tensor_tensor(tcym[:], tcy[:], mask[:], op=OP.mult)
                whmx = asm_pool.tile([128, G], F32, tag="whmx")
                nc.vector.scalar_tensor_tensor(whmx[:], wh0[:], 0.5 / W, mask[:],
                                               op0=OP.mult, op1=OP.mult)
                whmy = asm_pool.tile([128, G], F32, tag="whmy")
                nc.vector.scalar_tensor_tensor(whmy[:], wh1[:], 0.5 / H, mask[:],
                                               op0=OP.mult, op1=OP.mult)

                nc.vector.tensor_tensor(o3[:, :, 0], tcxm[:], whmx[:], op=OP.subtract)
                nc.vector.tensor_tensor(o3[:, :, 1], tcym[:], whmy[:], op=OP.subtract)
                nc.vector.tensor_tensor(o3[:, :, 2], tcxm[:], whmx[:], op=OP.add)
                nc.vector.tensor_tensor(o3[:, :, 3], tcym[:], whmy[:], op=OP.add)
                nc.vector.tensor_tensor(o3[:, :, 4], confs[:], mask[:], op=OP.mult)
                nc.vector.tensor_tensor(o3[:, :, 5], ig[:], mask[:], op=OP.mult)

                nc.sync.dma_start(
                    dets[b].rearrange("(p g) k -> p (g k)", p=128), out_img[:])

    nc.compile()
    return nc


_CACHE = {}
_CACHE_LOCK = threading.Lock()


def _get_nc(key, **kw):
    with _CACHE_LOCK:
        if key not in _CACHE:
            _CACHE[key] = build_nc(**kw)
        return _CACHE[key]


def _xyv(H, W):
    yv, xv = np.meshgrid(np.arange(H, dtype=np.float32),
                         np.arange(W, dtype=np.float32), indexing="ij")
    return np.stack([xv / W, yv / H]).reshape(2, H * W).astype(np.float32)


def kernel(hm: np.ndarray, wh: np.ndarray, reg: np.ndarray) -> np.ndarray:
    from concourse.bass_utils import run_bass_kernel_spmd

    B, C, H, W = hm.shape
    n_cores = 8
    assert B % n_cores == 0
    Bc = B // n_cores
    nc = _get_nc(("full", Bc, C, H, W), Bc=Bc, C=C, H=H, W=W, S=16)
    xyv = _xyv(H, W)
    in_maps = []
    for i in range(n_cores):
        sl = slice(i * Bc, (i + 1) * Bc)
        in_maps.append({
            "hm": np.ascontiguousarray(hm[sl]),
            "wh": np.ascontiguousarray(wh[sl]).reshape(Bc, 2, H * W),
            "reg": np.ascontiguousarray(reg[sl]).reshape(Bc, 2, H * W),
            "xyv": xyv,
        })
    res = run_bass_kernel_spmd(nc, in_maps, core_ids=list(range(n_cores)))
    return np.concatenate([res.results[i]["dets"] for i in range(n_cores)],
                          axis=0)
